# revision 37
# baseline (speedup 1.0000x reference)
"""Trainium2 8-core kernel for an attention block (per-head full-width QKV).

Reference computation (B=2, S=2048, H=12, D=768):
    Q/K/V = einsum('bsd,hde->bhse', x, W_{q,k,v})      # per-head D->D projections
    attn  = causal softmax(Q K^T / sqrt(D)) @ V
    out   = concat_heads(attn) @ W_o.T                 # [B,S,D]
    out   = out + gelu(LN(out) @ ff_w1.T) @ ff_w2.T

Sharding over 8 cores: 2 batch groups x 4 ranks. Core c = 4*b + r handles
batch b and heads [3r, 3r+3). Each core computes its heads' attention and
partial W_o product, a ReduceScatter over the 4-core group sums the partials
and hands each rank a 512-row sequence slice, on which the core runs
LN + FFN + residual. The host gathers the 8 [512, 768] outputs.

All matmuls run in bf16 (f32 PSUM accumulation); softmax / layernorm
statistics are kept in f32. Softmax is computed without max-subtraction
(scores for this problem are O(1)) and normalization is deferred past the
attn@V matmul: ctx_unnorm / rowsum(exp).
"""

import math
from dataclasses import dataclass

import numpy as np
import ml_dtypes

P = 128
SL = 512  # q-chunk width (PSUM bank / matmul free-dim limit)


@dataclass(frozen=True)
class Cfg:
    S: int = 2048          # sequence length
    D: int = 768           # model dim (= per-head dim here)
    FF: int = 3072         # FFN hidden dim
    HEADS: int = 3         # heads per core
    R: int = 4             # ranks per reduce-scatter group
    n_cores: int = 8

    @property
    def dch(self):
        return self.D // P

    @property
    def fch(self):
        return self.FF // P

    @property
    def qc(self):
        return self.S // SL

    @property
    def kt(self):
        return self.S // P

    @property
    def q_local(self):
        return self.S // self.R

    @property
    def qlt(self):
        return self.q_local // P


def build_graph(cfg: Cfg, no_collective: bool = False):
    """no_collective=True replaces the ReduceScatter with a local DMA so the
    graph can run under the single-core TimelineSim for perf iteration."""
    import concourse.tile as tile
    from concourse import bacc, mybir
    from concourse.masks import make_identity

    f32 = mybir.dt.float32
    bf16 = mybir.dt.bfloat16
    S, D, FF = cfg.S, cfg.D, cfg.FF
    DCH, FCH, QC, KT, QLT = cfg.dch, cfg.fch, cfg.qc, cfg.kt, cfg.qlt
    HEADS, R = cfg.HEADS, cfg.R
    DP = SL // P  # k-tiles per q-chunk on the diagonal (4)
    # split the D free-dim into <=SL pieces for matmuls (PSUM bank limit)
    d_splits = [(s0, min(s0 + SL, D)) for s0 in range(0, D, SL)]
    inv_sqrt_d = 1.0 / math.sqrt(D)
    n_groups = cfg.n_cores // R
    replica_groups = [list(range(g * R, (g + 1) * R)) for g in range(n_groups)]

    nc = bacc.Bacc(
        "TRN2",
        target_bir_lowering=False,
        debug=False,
        enable_asserts=True,
        num_devices=cfg.n_cores,
    )

    # ---- I/O (per-core shards, provided pre-transposed / pre-cast by host) ----
    x_t = nc.dram_tensor("x_t", [D, S], bf16, kind="ExternalInput")          # x[b].T
    w_q = nc.dram_tensor("w_q", [HEADS, D, D], bf16, kind="ExternalInput")   # [h, d, e]
    w_k = nc.dram_tensor("w_k", [HEADS, D, D], bf16, kind="ExternalInput")
    w_v = nc.dram_tensor("w_v", [HEADS, D, D], bf16, kind="ExternalInput")
    w_o_t = nc.dram_tensor("w_o_t", [HEADS * D, D], bf16, kind="ExternalInput")  # W_o slice.T
    ff_w1_t = nc.dram_tensor("ff_w1_t", [D, FF], bf16, kind="ExternalInput")     # ff_w1.T
    ff_w2_t = nc.dram_tensor("ff_w2_t", [FF, D], bf16, kind="ExternalInput")     # ff_w2.T
    out_ext = nc.dram_tensor("out", [cfg.q_local, D], f32, kind="ExternalOutput")

    with tile.TileContext(nc) as tc:
        with (
            tc.tile_pool(name="consts", bufs=1) as consts,
            tc.tile_pool(name="big", bufs=1) as big,
            tc.tile_pool(name="wts", bufs=1) as wts,
            tc.tile_pool(name="wo_w", bufs=1) as wo_w,
            tc.tile_pool(name="attn", bufs=2) as attn_pool,
            tc.tile_pool(name="small", bufs=2) as small,
            tc.tile_pool(name="stage", bufs=2) as stage,
            tc.tile_pool(name="dram", bufs=1, space="DRAM") as dram_pool,
            tc.tile_pool(name="psA", bufs=2, space="PSUM") as psA,
            tc.tile_pool(name="psB", bufs=6, space="PSUM") as psB,
        ):
            # internal DRAM for the reduce-scatter
            rs_in = dram_pool.tile([S, D], f32, name="rs_in")
            rs_out = dram_pool.tile([cfg.q_local, D], f32, name="rs_out")

            # ---- constants ----
            # one wide causal mask; the DP diagonal [128 k, 512 q] tile masks
            # (mask_m[kr, qc] = 1 where qc - kr - 128*m >= 0) are shifted views
            mask_w = (DP - 1) * P + SL
            mask_wide = consts.tile([P, mask_w], bf16, tag="mask", name="mask_wide")
            nc.gpsimd.memset(mask_wide, 1.0)
            nc.gpsimd.affine_select(
                out=mask_wide,
                in_=mask_wide,
                compare_op=mybir.AluOpType.is_ge,
                fill=0.0,
                base=-(DP - 1) * P,
                pattern=[[1, mask_w]],
                channel_multiplier=-1,
            )
            masks = [
                mask_wide[:, (DP - 1 - m) * P:(DP - 1 - m) * P + SL]
                for m in range(DP)
            ]
            identity = consts.tile([P, P], bf16, tag="ident", name="identity")
            make_identity(nc, identity)
            identity_f = consts.tile([P, P], f32, tag="identf", name="identity_f")
            make_identity(nc, identity_f)
            eps_col = consts.tile([P, 1], f32, tag="eps", name="eps_col")
            nc.vector.memset(eps_col, 1e-5)

            def load_head_weights(h):
                wq_h = wts.tile([P, DCH, D], bf16, tag="wq", name=f"wq{h}")
                wk_h = wts.tile([P, DCH, D], bf16, tag="wk", name=f"wk{h}")
                wv_h = wts.tile([P, DCH, D], bf16, tag="wv", name=f"wv{h}")
                wo_h = wo_w.tile([P, DCH, D], bf16, tag="wo", name=f"wo{h}")
                nc.sync.dma_start(wq_h, w_q.ap()[h].rearrange("(c p) e -> p c e", p=P))
                nc.sync.dma_start(wk_h, w_k.ap()[h].rearrange("(c p) e -> p c e", p=P))
                nc.sync.dma_start(wv_h, w_v.ap()[h].rearrange("(c p) e -> p c e", p=P))
                nc.sync.dma_start(
                    wo_h,
                    w_o_t.ap()[h * D:(h + 1) * D, :].rearrange(
                        "(c p) e -> p c e", p=P
                    ),
                )
                return wq_h, wk_h, wv_h, wo_h

            def emit_wo_block(ctxn_p, rec_p, wo_p, base_q, is_first_head):
                for ql in range(DP):
                    q0 = base_q + ql * P
                    wops = [
                        psB.tile([P, SL], f32, tag="psB", name=f"wo{i}")
                        for i in range(len(d_splits))
                    ]
                    for jc in range(DCH):
                        for wo_ps, (e0, e1) in zip(wops, d_splits):
                            nc.tensor.matmul(
                                wo_ps[:, : e1 - e0],
                                ctxn_p[:, jc, ql * P:(ql + 1) * P],
                                wo_p[:, jc, e0:e1],
                                start=(jc == 0),
                                stop=(jc == DCH - 1),
                            )
                    wo_stage = stage.tile(
                        [P, D], f32, tag="st768", bufs=1, name="wo_stage"
                    )
                    for wo_ps, (e0, e1) in zip(wops, d_splits):
                        nc.vector.tensor_scalar_mul(
                            out=wo_stage[:, e0:e1],
                            in0=wo_ps[:, : e1 - e0],
                            scalar1=rec_p[:, ql:ql + 1],
                        )
                    if is_first_head:
                        nc.sync.dma_start(out=rs_in[q0:q0 + P, :], in_=wo_stage)
                    else:
                        nc.gpsimd.dma_start(
                            out=rs_in[q0:q0 + P, :],
                            in_=wo_stage,
                            accum_op=mybir.AluOpType.add,
                        )

            pending_wo = None

            # startup: interleave head-0 weight DMAs with x^T chunks so the
            # first projection (needs wq + x chunk 0 only) starts ASAP
            xt = big.tile([P, DCH, S], bf16, tag="xt", name="xt")
            xt_src = x_t.ap().rearrange("(c p) s -> p c s", p=P)
            wq_0 = wts.tile([P, DCH, D], bf16, tag="wq", name="wq0")
            wk_0 = wts.tile([P, DCH, D], bf16, tag="wk", name="wk0")
            wv_0 = wts.tile([P, DCH, D], bf16, tag="wv", name="wv0")
            wo_0 = wo_w.tile([P, DCH, D], bf16, tag="wo", name="wo0")
            nc.sync.dma_start(wq_0, w_q.ap()[0].rearrange("(c p) e -> p c e", p=P))
            nc.sync.dma_start(xt[:, :, 0:SL], xt_src[:, :, 0:SL])
            nc.sync.dma_start(wk_0, w_k.ap()[0].rearrange("(c p) e -> p c e", p=P))
            for sc in range(1, QC):
                nc.sync.dma_start(
                    xt[:, :, sc * SL:(sc + 1) * SL],
                    xt_src[:, :, sc * SL:(sc + 1) * SL],
                )
            nc.sync.dma_start(wv_0, w_v.ap()[0].rearrange("(c p) e -> p c e", p=P))
            nc.sync.dma_start(
                wo_0, w_o_t.ap()[0:D, :].rearrange("(c p) e -> p c e", p=P)
            )
            head_weights = (wq_0, wk_0, wv_0, wo_0)

            for h in range(HEADS):
                wq_h, wk_h, wv_h, wo_h = (
                    head_weights if h == 0 else load_head_weights(h)
                )

                # ---- projections: Q^T, K^T [e, s], V [s, e] (bf16) ----
                qt = big.tile([P, DCH, S], bf16, tag="qt", name=f"qt{h}")
                kt_sb = big.tile([P, DCH, S], bf16, tag="kt", name=f"kt{h}")
                v_sb = big.tile([P, KT, D], bf16, tag="v", name=f"v{h}")

                for tgt, w_h in ((qt, wq_h), (kt_sb, wk_h)):
                    for sc in range(QC):
                        for ec in range(DCH):
                            ps = psA.tile([P, SL], f32, tag="psA", name="ps_proj")
                            for dc in range(DCH):
                                nc.tensor.matmul(
                                    ps,
                                    w_h[:, dc, ec * P:(ec + 1) * P],
                                    xt[:, dc, sc * SL:(sc + 1) * SL],
                                    start=(dc == 0),
                                    stop=(dc == DCH - 1),
                                )
                            nc.vector.tensor_copy(
                                out=tgt[:, ec, sc * SL:(sc + 1) * SL], in_=ps
                            )
                for kti in range(KT):
                    pvs = [
                        psB.tile([P, SL], f32, tag="psB", name=f"pv{i}")
                        for i in range(len(d_splits))
                    ]
                    for dc in range(DCH):
                        for pv, (e0, e1) in zip(pvs, d_splits):
                            nc.tensor.matmul(
                                pv[:, : e1 - e0],
                                xt[:, dc, kti * P:(kti + 1) * P],
                                wv_h[:, dc, e0:e1],
                                start=(dc == 0),
                                stop=(dc == DCH - 1),
                            )
                    for pv, (e0, e1) in zip(pvs, d_splits):
                        nc.vector.tensor_copy(
                            out=v_sb[:, kti, e0:e1], in_=pv[:, : e1 - e0]
                        )

                # ---- attention, one q-chunk (512 queries) at a time ----
                for sc in range(QC):
                    ctxn = big.tile(
                        [P, DCH, SL], bf16, tag="ctxn", bufs=2, name=f"ctxn{h}_{sc}"
                    )
                    n_kt = (sc + 1) * DP  # causal: k tiles 0 .. n_kt-1
                    diag0 = sc * DP       # first diagonal k-tile index
                    ctx_ps = [
                        psB.tile([P, SL], f32, tag="psB", name=f"ctx_ps{ec}")
                        for ec in range(DCH)
                    ]
                    dacc = small.tile([P, SL], f32, tag="dacc", bufs=1, name="dacc")
                    for kti in range(n_kt):
                        # diagonal tiles only need queries q >= k: narrow the
                        # q range to [o, SL) instead of masking full width
                        m = kti - diag0
                        o = m * P if m > 0 else 0
                        w = SL - o
                        st_ps = psA.tile([P, SL], f32, tag="psA", name="st_ps")
                        for ec in range(DCH):
                            nc.tensor.matmul(
                                st_ps[:, :w],
                                kt_sb[:, ec, kti * P:(kti + 1) * P],
                                qt[:, ec, sc * SL + o:(sc + 1) * SL],
                                start=(ec == 0),
                                stop=(ec == DCH - 1),
                            )
                        es = attn_pool.tile([P, SL], bf16, tag="es", name="es")
                        nc.scalar.activation(
                            out=es[:, :w],
                            in_=st_ps[:, :w],
                            func=mybir.ActivationFunctionType.Exp,
                            scale=inv_sqrt_d,
                        )
                        if m >= 0:
                            nc.vector.tensor_mul(
                                out=es[:, :w], in0=es[:, :w], in1=masks[0][:, :w]
                            )
                        if kti == 0:
                            nc.vector.tensor_copy(out=dacc, in_=es)
                        else:
                            nc.vector.tensor_add(
                                out=dacc[:, o:], in0=dacc[:, o:], in1=es[:, :w]
                            )
                        for ec in range(DCH):
                            nc.tensor.matmul(
                                ctx_ps[ec][:, o:],
                                v_sb[:, kti, ec * P:(ec + 1) * P],
                                es[:, :w],
                                start=(kti == 0),
                                stop=(kti == n_kt - 1),
                                skip_group_check=True,
                            )
                    # copy ctx PSUM -> SBUF bf16, alternating Scalar/Vector
                    # engines (frees the psB banks; overlaps PE work)
                    for jc in range(DCH):
                        if jc % 2 == 0:
                            nc.scalar.copy(out=ctxn[:, jc, :], in_=ctx_ps[jc])
                        else:
                            nc.vector.tensor_copy(out=ctxn[:, jc, :], in_=ctx_ps[jc])

                    # W_o of the PREVIOUS chunk: its inputs are long ready, so
                    # this keeps PE busy while this chunk's ctx copies /
                    # denominator chain resolve (software pipeline, depth 1)
                    if pending_wo is not None:
                        emit_wo_block(*pending_wo)

                    # softmax denominator -> per-partition scalars in q
                    # orientation: transpose dacc 128x128 blocks on PE, then
                    # free-axis reduce. 1/denom is applied at the W_o stage.
                    # (emitted after the W_o block: by then dacc is long done,
                    # so the PE transposes don't wait on the DVE accumulator)
                    rec_d = small.tile([P, DP], f32, tag="rec", name="rec_d")
                    for ql in range(DP):
                        dt_ps = psA.tile([P, P], f32, tag="psA", name="dt_ps")
                        nc.tensor.transpose(
                            dt_ps, dacc[:, ql * P:(ql + 1) * P], identity_f
                        )
                        nc.vector.reduce_sum(
                            out=rec_d[:, ql:ql + 1],
                            in_=dt_ps,
                            axis=mybir.AxisListType.X,
                        )
                    nc.vector.reciprocal(out=rec_d, in_=rec_d)
                    pending_wo = (ctxn, rec_d, wo_h, sc * SL, h == 0)

            # flush the final pipelined W_o block before the reduce-scatter
            if pending_wo is not None:
                emit_wo_block(*pending_wo)
                pending_wo = None

            # ---- reduce-scatter: sum partials over the group, keep local rows ----
            if no_collective:
                nc.sync.dma_start(out=rs_out, in_=rs_in[: cfg.q_local, :])
            else:
                nc.gpsimd.collective_compute(
                    "ReduceScatter",
                    mybir.AluOpType.add,
                    replica_groups=replica_groups,
                    ins=[rs_in.opt()],
                    outs=[rs_out.opt()],
                )

            # ---- FFN on the local q_local rows ----
            # ff_w2 stays resident; ff_w1 is streamed per 128-wide f-chunk
            ffw2 = wts.tile([P, FCH, D], bf16, tag="ffw2", name="ffw2")
            nc.sync.dma_start(ffw2, ff_w2_t.ap().rearrange("(c p) e -> p c e", p=P))

            resid = big.tile([P, QLT, D], f32, tag="v", name="resid")
            nc.sync.dma_start(resid, rs_out.rearrange("(t p) e -> p t e", p=P))

            # layernorm (no affine) -> ln^T bf16 [d, q_local]
            # stats for all q-tiles first, then the transposes, so PE streams
            # through the transposes without per-tile DVE round trips
            lnT = big.tile([P, DCH, cfg.q_local], bf16, tag="xt", name="lnT")
            ln_all = stage.tile([P, QLT, D], bf16, tag="ln_row", bufs=1, name="ln_all")
            for qt_i in range(QLT):
                x_row = resid[:, qt_i, :]
                sub = math.gcd(512, D)
                nsub = D // sub
                stats = small.tile([P, nsub, 6], f32, tag="stats", name="stats")
                for si in range(nsub):
                    nc.vector.bn_stats(
                        out=stats[:, si, :], in_=x_row[:, si * sub:(si + 1) * sub]
                    )
                mv = small.tile([P, 2], f32, tag="mv", name="mv")
                nc.vector.bn_aggr(out=mv, in_=stats)
                rstd = small.tile([P, 1], f32, tag="rstd", name="rstd")
                nc.scalar.activation(
                    out=rstd,
                    in_=mv[:, 1:2],
                    func=mybir.ActivationFunctionType.Sqrt,
                    bias=eps_col,
                    scale=1.0,
                )
                nc.vector.reciprocal(out=rstd, in_=rstd)
                nc.vector.tensor_scalar(
                    out=ln_all[:, qt_i, :],
                    in0=x_row,
                    scalar1=mv[:, 0:1],
                    scalar2=rstd,
                    op0=mybir.AluOpType.subtract,
                    op1=mybir.AluOpType.mult,
                )
            for qt_i in range(QLT):
                for dc in range(DCH):
                    tr_ps = psA.tile([P, P], bf16, tag="psA", name="tr_ps")
                    nc.tensor.transpose(
                        tr_ps, ln_all[:, qt_i, dc * P:(dc + 1) * P], identity
                    )
                    nc.vector.tensor_copy(
                        out=lnT[:, dc, qt_i * P:(qt_i + 1) * P], in_=tr_ps
                    )

            # h^T = gelu(ff_w1 @ ln^T)  [f, q_local] bf16
            hT = big.tile([P, FCH, cfg.q_local], bf16, tag="qt", name="hT")
            QS = min(SL, cfg.q_local)
            for fc in range(FCH):
                ffw1_fc = wts.tile([P, DCH, P], bf16, tag="ffw1c", bufs=4,
                                   name=f"ffw1c{fc}")
                nc.sync.dma_start(
                    ffw1_fc,
                    ff_w1_t.ap()[:, fc * P:(fc + 1) * P].rearrange(
                        "(c p) f -> p c f", p=P
                    ),
                )
                for qs in range(cfg.q_local // QS):
                    hp = psB.tile([P, SL], f32, tag="psB", name="hp")
                    for dc in range(DCH):
                        nc.tensor.matmul(
                            hp[:, :QS],
                            ffw1_fc[:, dc, :],
                            lnT[:, dc, qs * QS:(qs + 1) * QS],
                            start=(dc == 0),
                            stop=(dc == DCH - 1),
                        )
                    nc.scalar.activation(
                        out=hT[:, fc, qs * QS:(qs + 1) * QS],
                        in_=hp[:, :QS],
                        func=mybir.ActivationFunctionType.Gelu,
                        scale=1.0,
                    )

            # y = h^T.T @ ff_w2^T + resid -> out
            for qt_i in range(QLT):
                yps = [
                    psB.tile([P, SL], f32, tag="psB", name=f"y{i}")
                    for i in range(len(d_splits))
                ]
                for fc in range(FCH):
                    for y_ps, (e0, e1) in zip(yps, d_splits):
                        nc.tensor.matmul(
                            y_ps[:, : e1 - e0],
                            hT[:, fc, qt_i * P:(qt_i + 1) * P],
                            ffw2[:, fc, e0:e1],
                            start=(fc == 0),
                            stop=(fc == FCH - 1),
                        )
                out_stage = stage.tile([P, D], f32, tag="st768", bufs=1, name="out_stage")
                for y_ps, (e0, e1) in zip(yps, d_splits):
                    nc.vector.tensor_add(
                        out=out_stage[:, e0:e1],
                        in0=y_ps[:, : e1 - e0],
                        in1=resid[:, qt_i, e0:e1],
                    )
                nc.sync.dma_start(
                    out=out_ext.ap()[qt_i * P:(qt_i + 1) * P, :], in_=out_stage
                )

    nc.compile()
    return nc


def shard_inputs(x, W_q, W_k, W_v, W_o, ff_w1, ff_w2, cfg: Cfg):
    bf16 = ml_dtypes.bfloat16
    in_maps = []
    D = cfg.D
    for c in range(cfg.n_cores):
        b, r = divmod(c, cfg.R)
        hs = slice(cfg.HEADS * r, cfg.HEADS * (r + 1))
        j0 = cfg.HEADS * D * r
        in_maps.append(
            {
                "x_t": np.ascontiguousarray(x[b].T).astype(bf16),
                "w_q": np.ascontiguousarray(W_q[hs]).astype(bf16),
                "w_k": np.ascontiguousarray(W_k[hs]).astype(bf16),
                "w_v": np.ascontiguousarray(W_v[hs]).astype(bf16),
                "w_o_t": np.ascontiguousarray(
                    W_o[:, j0:j0 + cfg.HEADS * D].T
                ).astype(bf16),
                "ff_w1_t": np.ascontiguousarray(ff_w1.T).astype(bf16),
                "ff_w2_t": np.ascontiguousarray(ff_w2.T).astype(bf16),
            }
        )
    return in_maps


def gather_outputs(results, cfg: Cfg, B):
    out = np.zeros((B, cfg.S, cfg.D), np.float32)
    for c in range(cfg.n_cores):
        b, r = divmod(c, cfg.R)
        out[b, cfg.q_local * r:cfg.q_local * (r + 1), :] = results[c]["out"]
    return out


def kernel(x, W_q, W_k, W_v, W_o, ff_w1, ff_w2):
    import sys

    if "/opt/trn_rl_repo" not in sys.path:
        sys.path.insert(0, "/opt/trn_rl_repo")
    from concourse.bass_utils import run_bass_kernel_spmd

    cfg = Cfg()
    nc = build_graph(cfg)
    in_maps = shard_inputs(x, W_q, W_k, W_v, W_o, ff_w1, ff_w2, cfg)
    res = run_bass_kernel_spmd(nc, in_maps, core_ids=list(range(cfg.n_cores)))
    return gather_outputs(res.results, cfg, x.shape[0])


# revision 54
# speedup vs baseline: 6001.3323x; 6001.3323x over previous
"""Trainium2 8-core kernel for an attention block (per-head full-width QKV).

Reference computation (B=2, S=2048, H=12, D=768):
    Q/K/V = einsum('bsd,hde->bhse', x, W_{q,k,v})      # per-head D->D projections
    attn  = causal softmax(Q K^T / sqrt(D)) @ V
    out   = concat_heads(attn) @ W_o.T                 # [B,S,D]
    out   = out + gelu(LN(out) @ ff_w1.T) @ ff_w2.T

Sharding over 8 cores: 2 batch groups x 4 ranks. Core c = 4*b + r handles
batch b and heads [3r, 3r+3). A ReduceScatter over each 4-core group sums the
per-head output partials and hands each rank a 512-row sequence slice, on
which the core runs LN + FFN + residual. The host gathers the 8 [512, 768]
outputs.

Key algebraic restructure: the per-head weight pairs are folded on the host,
    M_h = W_q[h] @ W_k[h].T        -> scores = x M_h x^T / sqrt(D)
    N_h = W_v[h] @ W_o[:, hD:+D].T -> out_h  = softmax_num @ (x N_h) / denom
which removes the K/V-vs-Q distinction (x^T itself is the score matmul's
stationary operand), the separate W_o stage, and one projection per head.
u = x N_h carries a trailing ones column, so the attn@u matmul produces the
softmax denominator on the same q partitions as the numerator (softmax is
computed without max-subtraction — scores here are O(1) — and normalization
happens after the k-sum).

All matmuls run in bf16 (f32 PSUM accumulation); softmax / layernorm
statistics are kept in f32.
"""

import math
from dataclasses import dataclass

import numpy as np
import ml_dtypes

P = 128
SL = 512  # q-chunk width (PSUM bank / matmul free-dim limit)


@dataclass(frozen=True)
class Cfg:
    S: int = 2048          # sequence length
    D: int = 768           # model dim (= per-head dim here)
    FF: int = 3072         # FFN hidden dim
    HEADS: int = 3         # heads per core
    R: int = 4             # ranks per reduce-scatter group
    n_cores: int = 8

    @property
    def dch(self):
        return self.D // P

    @property
    def fch(self):
        return self.FF // P

    @property
    def qc(self):
        return self.S // SL

    @property
    def kt(self):
        return self.S // P

    @property
    def q_local(self):
        return self.S // self.R

    @property
    def qlt(self):
        return self.q_local // P


def build_graph(cfg: Cfg, no_collective: bool = False):
    """no_collective=True replaces the ReduceScatter with a local DMA so the
    graph can run under the single-core TimelineSim for perf iteration."""
    import concourse.tile as tile
    from concourse import bacc, mybir
    from concourse.masks import make_identity

    f32 = mybir.dt.float32
    bf16 = mybir.dt.bfloat16
    S, D, FF = cfg.S, cfg.D, cfg.FF
    DCH, FCH, QC, KT, QLT = cfg.dch, cfg.fch, cfg.qc, cfg.kt, cfg.qlt
    HEADS, R = cfg.HEADS, cfg.R
    DP = SL // P  # k-tiles per q-chunk on the diagonal (4)
    # split the D free-dim into <=SL pieces for matmuls (PSUM bank limit)
    d_splits = [(s0, min(s0 + SL, D)) for s0 in range(0, D, SL)]
    # same for the u matrix, which has a trailing ones column (D+1 wide)
    u_splits = [(s0, min(s0 + SL, D + 1)) for s0 in range(0, D + 1, SL)]
    inv_sqrt_d = 1.0 / math.sqrt(D)
    n_groups = cfg.n_cores // R
    replica_groups = [list(range(g * R, (g + 1) * R)) for g in range(n_groups)]

    nc = bacc.Bacc(
        "TRN2",
        target_bir_lowering=False,
        debug=False,
        enable_asserts=True,
        num_devices=cfg.n_cores,
    )

    # ---- I/O (per-core shards, provided pre-transposed / pre-cast by host) ----
    x_t = nc.dram_tensor("x_t", [D, S], bf16, kind="ExternalInput")          # x[b].T
    # folded per-head weights (host-computed):
    #   m_w[h] = W_q[h] @ W_k[h].T            (scores = x M x^T / sqrt(D))
    #   n_w[h] = W_v[h] @ W_o[:, hD:(h+1)D].T (out_h  = softmax_num @ (x N))
    m_w = nc.dram_tensor("m_w", [HEADS, D, D], bf16, kind="ExternalInput")
    n_w = nc.dram_tensor("n_w", [HEADS, D, D], bf16, kind="ExternalInput")
    ff_w1_t = nc.dram_tensor("ff_w1_t", [D, FF], bf16, kind="ExternalInput")     # ff_w1.T
    ff_w2_t = nc.dram_tensor("ff_w2_t", [FF, D], bf16, kind="ExternalInput")     # ff_w2.T
    out_ext = nc.dram_tensor("out", [cfg.q_local, D], f32, kind="ExternalOutput")

    with tile.TileContext(nc) as tc:
        with (
            tc.tile_pool(name="consts", bufs=1) as consts,
            tc.tile_pool(name="big", bufs=1) as big,
            tc.tile_pool(name="wts", bufs=1) as wts,
            tc.tile_pool(name="attn", bufs=2) as attn_pool,
            tc.tile_pool(name="small", bufs=2) as small,
            tc.tile_pool(name="stage", bufs=2) as stage,
            tc.tile_pool(name="dram", bufs=1, space="DRAM") as dram_pool,
            tc.tile_pool(name="psA", bufs=2, space="PSUM") as psA,
            tc.tile_pool(name="psB", bufs=6, space="PSUM") as psB,
        ):
            # internal DRAM for the reduce-scatter
            rs_in = dram_pool.tile([S, D], f32, name="rs_in")
            rs_out = dram_pool.tile([cfg.q_local, D], f32, name="rs_out")

            # ---- constants ----
            # causal mask for the (narrowed) diagonal tiles:
            # mask0[kr, qc] = 1 where qc >= kr
            mask0 = consts.tile([P, SL], bf16, tag="mask", name="mask0")
            nc.gpsimd.memset(mask0, 1.0)
            nc.gpsimd.affine_select(
                out=mask0,
                in_=mask0,
                compare_op=mybir.AluOpType.is_ge,
                fill=0.0,
                base=0,
                pattern=[[1, SL]],
                channel_multiplier=-1,
            )
            masks = [mask0]
            identity = consts.tile([P, P], bf16, tag="ident", name="identity")
            make_identity(nc, identity)
            eps_col = consts.tile([P, 1], f32, tag="eps", name="eps_col")
            nc.vector.memset(eps_col, 1e-5)

            def load_head_weights(h, interleave_xt=False):
                mw_h = wts.tile([P, DCH, D], bf16, tag="mw", bufs=2, name=f"mw{h}")
                nw_h = wts.tile([P, DCH, D], bf16, tag="nw", bufs=2, name=f"nw{h}")
                mw_src = m_w.ap()[h].rearrange("(c p) e -> p c e", p=P)
                if interleave_xt:
                    # critical path at startup: the first projection group
                    # needs only m_w[:, :, 0:128] + x chunk 0 — load those
                    # first, then the rest
                    nc.sync.dma_start(mw_h[:, :, 0:P], mw_src[:, :, 0:P])
                    nc.sync.dma_start(xt[:, :, 0:SL], xt_src[:, :, 0:SL])
                    nc.sync.dma_start(mw_h[:, :, P:D], mw_src[:, :, P:D])
                else:
                    nc.sync.dma_start(mw_h, mw_src)
                nc.sync.dma_start(nw_h, n_w.ap()[h].rearrange("(c p) e -> p c e", p=P))
                return mw_h, nw_h

            xt = big.tile([P, DCH, S], bf16, tag="xt", name="xt")
            xt_src = x_t.ap().rearrange("(c p) s -> p c s", p=P)
            head_weights = load_head_weights(0, interleave_xt=True)
            for sc in range(1, QC):
                nc.sync.dma_start(
                    xt[:, :, sc * SL:(sc + 1) * SL],
                    xt_src[:, :, sc * SL:(sc + 1) * SL],
                )

            for h in range(HEADS):
                mw_h, nw_h = head_weights if h == 0 else load_head_weights(h)

                # ---- G^T = (M^T x^T) [d2, s] and u = x N (+ ones col) [s, d+1] ----
                gt = big.tile([P, DCH, S], bf16, tag="qt", name=f"gt{h}")
                u_sb = big.tile([P, KT, D + 1], bf16, tag="v", name=f"u{h}")
                nc.vector.memset(u_sb[:, :, D:D + 1], 1.0)

                for sc in range(QC):
                    for ec in range(DCH):
                        ps = psA.tile([P, SL], f32, tag="psA", name="ps_proj")
                        for dc in range(DCH):
                            nc.tensor.matmul(
                                ps,
                                mw_h[:, dc, ec * P:(ec + 1) * P],
                                xt[:, dc, sc * SL:(sc + 1) * SL],
                                start=(dc == 0),
                                stop=(dc == DCH - 1),
                            )
                        nc.vector.tensor_copy(
                            out=gt[:, ec, sc * SL:(sc + 1) * SL], in_=ps
                        )
                for kti in range(KT):
                    pvs = [
                        psB.tile([P, SL], f32, tag="psB", name=f"pv{i}")
                        for i in range(len(d_splits))
                    ]
                    for dc in range(DCH):
                        for pv, (e0, e1) in zip(pvs, d_splits):
                            nc.tensor.matmul(
                                pv[:, : e1 - e0],
                                xt[:, dc, kti * P:(kti + 1) * P],
                                nw_h[:, dc, e0:e1],
                                start=(dc == 0),
                                stop=(dc == DCH - 1),
                            )
                    for pv, (e0, e1) in zip(pvs, d_splits):
                        nc.vector.tensor_copy(
                            out=u_sb[:, kti, e0:e1], in_=pv[:, : e1 - e0]
                        )

                # ---- attention, one q-chunk (512 queries) at a time ----
                for sc in range(QC):
                    n_kt = (sc + 1) * DP  # causal: k tiles 0 .. n_kt-1
                    diag0 = sc * DP       # first diagonal k-tile index
                    es_all = attn_pool.tile(
                        [P, n_kt, SL], bf16, tag="es", bufs=1, name=f"es{h}_{sc}"
                    )
                    # scores pass: S^T tiles -> exp -> es_all (masked on diag)
                    for kti in range(n_kt):
                        m = kti - diag0
                        o = m * P if m > 0 else 0
                        w = SL - o
                        st_ps = psA.tile([P, SL], f32, tag="psA", name="st_ps")
                        for dc in range(DCH):
                            nc.tensor.matmul(
                                st_ps[:, :w],
                                xt[:, dc, kti * P:(kti + 1) * P],
                                gt[:, dc, sc * SL + o:(sc + 1) * SL],
                                start=(dc == 0),
                                stop=(dc == DCH - 1),
                            )
                        nc.scalar.activation(
                            out=es_all[:, kti, :w],
                            in_=st_ps[:, :w],
                            func=mybir.ActivationFunctionType.Exp,
                            scale=inv_sqrt_d,
                        )
                        if m >= 0:
                            nc.vector.tensor_mul(
                                out=es_all[:, kti, :w],
                                in0=es_all[:, kti, :w],
                                in1=mask0[:, :w],
                            )
                    # numerator+denominator pass: out'[q,:] = sum_k es^T u'
                    # (u has a trailing ones column -> col D is the softmax
                    # denominator, landing on the q partitions directly).
                    # two q-subtiles at a time to fit PSUM.
                    for half in range(DP // 2):
                        qls = (2 * half, 2 * half + 1)
                        ops = {
                            ql: [
                                psB.tile([P, SL], f32, tag="psB", name=f"o{ql}_{i}")
                                for i in range(len(u_splits))
                            ]
                            for ql in qls
                        }
                        for kti in range(n_kt):
                            m = kti - diag0
                            o = m * P if m > 0 else 0
                            for ql in qls:
                                if m > ql:
                                    continue  # fully masked block
                                es_sl = es_all[:, kti, ql * P - o:(ql + 1) * P - o]
                                for op_t, (e0, e1) in zip(ops[ql], u_splits):
                                    nc.tensor.matmul(
                                        op_t[:, : e1 - e0],
                                        es_sl,
                                        u_sb[:, kti, e0:e1],
                                        start=(kti == 0),
                                        stop=(kti == diag0 + ql),
                                        skip_group_check=True,
                                    )
                        for ql in qls:
                            q0 = sc * SL + ql * P
                            last_e0 = u_splits[-1][0]
                            recd = small.tile([P, 1], f32, tag="recd", name="recd")
                            nc.vector.reciprocal(
                                out=recd,
                                in_=ops[ql][-1][:, D - last_e0:D - last_e0 + 1],
                            )
                            wo_stage = stage.tile(
                                [P, D], f32, tag="st768", bufs=1, name="wo_stage"
                            )
                            for op_t, (e0, e1) in zip(ops[ql], u_splits):
                                nc.vector.tensor_scalar_mul(
                                    out=wo_stage[:, e0:min(e1, D)],
                                    in0=op_t[:, : min(e1, D) - e0],
                                    scalar1=recd,
                                )
                            if h == 0:
                                nc.sync.dma_start(
                                    out=rs_in[q0:q0 + P, :], in_=wo_stage
                                )
                            else:
                                nc.gpsimd.dma_start(
                                    out=rs_in[q0:q0 + P, :],
                                    in_=wo_stage,
                                    accum_op=mybir.AluOpType.add,
                                )

            # ---- reduce-scatter: sum partials over the group, keep local rows ----
            if no_collective:
                nc.sync.dma_start(out=rs_out, in_=rs_in[: cfg.q_local, :])
            else:
                nc.gpsimd.collective_compute(
                    "ReduceScatter",
                    mybir.AluOpType.add,
                    replica_groups=replica_groups,
                    ins=[rs_in.opt()],
                    outs=[rs_out.opt()],
                )

            # ---- FFN on the local q_local rows ----
            # ff_w2 stays resident; ff_w1 is streamed per 128-wide f-chunk
            ffw2 = wts.tile([P, FCH, D], bf16, tag="ffw2", name="ffw2")
            nc.sync.dma_start(ffw2, ff_w2_t.ap().rearrange("(c p) e -> p c e", p=P))

            # residual rows, one q-tile per DMA so LN stats start early
            resid = big.tile([P, QLT, D], f32, tag="v", name="resid")
            resid_src = rs_out.rearrange("(t p) e -> p t e", p=P)
            for qt_i in range(QLT):
                nc.sync.dma_start(
                    resid[:, qt_i, :], resid_src[:, qt_i, :]
                )

            # layernorm (no affine) -> ln^T bf16 [d, q_local]
            # stats for all q-tiles first, then the transposes, so PE streams
            # through the transposes without per-tile DVE round trips
            lnT = big.tile([P, DCH, cfg.q_local], bf16, tag="xt", name="lnT")
            ln_all = stage.tile([P, QLT, D], bf16, tag="ln_row", bufs=1, name="ln_all")
            for qt_i in range(QLT):
                x_row = resid[:, qt_i, :]
                sub = math.gcd(512, D)
                nsub = D // sub
                stats = small.tile([P, nsub, 6], f32, tag="stats", name="stats")
                for si in range(nsub):
                    nc.vector.bn_stats(
                        out=stats[:, si, :], in_=x_row[:, si * sub:(si + 1) * sub]
                    )
                mv = small.tile([P, 2], f32, tag="mv", name="mv")
                nc.vector.bn_aggr(out=mv, in_=stats)
                rstd = small.tile([P, 1], f32, tag="rstd", name="rstd")
                nc.scalar.activation(
                    out=rstd,
                    in_=mv[:, 1:2],
                    func=mybir.ActivationFunctionType.Sqrt,
                    bias=eps_col,
                    scale=1.0,
                )
                nc.vector.reciprocal(out=rstd, in_=rstd)
                nc.vector.tensor_scalar(
                    out=ln_all[:, qt_i, :],
                    in0=x_row,
                    scalar1=mv[:, 0:1],
                    scalar2=rstd,
                    op0=mybir.AluOpType.subtract,
                    op1=mybir.AluOpType.mult,
                )
            for qt_i in range(QLT):
                for dc in range(DCH):
                    tr_ps = psA.tile([P, P], bf16, tag="psA", name="tr_ps")
                    nc.tensor.transpose(
                        tr_ps, ln_all[:, qt_i, dc * P:(dc + 1) * P], identity
                    )
                    nc.vector.tensor_copy(
                        out=lnT[:, dc, qt_i * P:(qt_i + 1) * P], in_=tr_ps
                    )

            # h^T = gelu(ff_w1 @ ln^T)  [f, q_local] bf16
            hT = big.tile([P, FCH, cfg.q_local], bf16, tag="qt", name="hT")
            QS = min(SL, cfg.q_local)
            for fc in range(FCH):
                ffw1_fc = wts.tile([P, DCH, P], bf16, tag="ffw1c", bufs=4,
                                   name=f"ffw1c{fc}")
                nc.sync.dma_start(
                    ffw1_fc,
                    ff_w1_t.ap()[:, fc * P:(fc + 1) * P].rearrange(
                        "(c p) f -> p c f", p=P
                    ),
                )
                for qs in range(cfg.q_local // QS):
                    hp = psB.tile([P, SL], f32, tag="psB", name="hp")
                    for dc in range(DCH):
                        nc.tensor.matmul(
                            hp[:, :QS],
                            ffw1_fc[:, dc, :],
                            lnT[:, dc, qs * QS:(qs + 1) * QS],
                            start=(dc == 0),
                            stop=(dc == DCH - 1),
                        )
                    nc.scalar.activation(
                        out=hT[:, fc, qs * QS:(qs + 1) * QS],
                        in_=hp[:, :QS],
                        func=mybir.ActivationFunctionType.Gelu,
                        scale=1.0,
                    )

            # y = h^T.T @ ff_w2^T + resid -> out
            for qt_i in range(QLT):
                yps = [
                    psB.tile([P, SL], f32, tag="psB", name=f"y{i}")
                    for i in range(len(d_splits))
                ]
                for fc in range(FCH):
                    for y_ps, (e0, e1) in zip(yps, d_splits):
                        nc.tensor.matmul(
                            y_ps[:, : e1 - e0],
                            hT[:, fc, qt_i * P:(qt_i + 1) * P],
                            ffw2[:, fc, e0:e1],
                            start=(fc == 0),
                            stop=(fc == FCH - 1),
                        )
                out_stage = stage.tile([P, D], f32, tag="st768", bufs=1, name="out_stage")
                for y_ps, (e0, e1) in zip(yps, d_splits):
                    nc.vector.tensor_add(
                        out=out_stage[:, e0:e1],
                        in0=y_ps[:, : e1 - e0],
                        in1=resid[:, qt_i, e0:e1],
                    )
                nc.sync.dma_start(
                    out=out_ext.ap()[qt_i * P:(qt_i + 1) * P, :], in_=out_stage
                )

    nc.compile()
    return nc


def shard_inputs(x, W_q, W_k, W_v, W_o, ff_w1, ff_w2, cfg: Cfg):
    bf16 = ml_dtypes.bfloat16
    in_maps = []
    D = cfg.D
    ff1 = np.ascontiguousarray(ff_w1.T).astype(bf16)
    ff2 = np.ascontiguousarray(ff_w2.T).astype(bf16)
    for c in range(cfg.n_cores):
        b, r = divmod(c, cfg.R)
        heads = range(cfg.HEADS * r, cfg.HEADS * (r + 1))
        # fold the per-head weight pairs on the host (fp32, then bf16):
        #   m[h] = W_q[h] @ W_k[h].T ; n[h] = W_v[h] @ W_o[:, hD:(h+1)D].T
        m = np.stack([W_q[h] @ W_k[h].T for h in heads])
        n = np.stack(
            [W_v[h] @ W_o[:, h * D:(h + 1) * D].T for h in heads]
        )
        in_maps.append(
            {
                "x_t": np.ascontiguousarray(x[b].T).astype(bf16),
                "m_w": m.astype(bf16),
                "n_w": n.astype(bf16),
                "ff_w1_t": ff1,
                "ff_w2_t": ff2,
            }
        )
    return in_maps


def gather_outputs(results, cfg: Cfg, B):
    out = np.zeros((B, cfg.S, cfg.D), np.float32)
    for c in range(cfg.n_cores):
        b, r = divmod(c, cfg.R)
        out[b, cfg.q_local * r:cfg.q_local * (r + 1), :] = results[c]["out"]
    return out


def kernel(x, W_q, W_k, W_v, W_o, ff_w1, ff_w2):
    import sys

    if "/opt/trn_rl_repo" not in sys.path:
        sys.path.insert(0, "/opt/trn_rl_repo")
    from concourse.bass_utils import run_bass_kernel_spmd

    cfg = Cfg()
    nc = build_graph(cfg)
    in_maps = shard_inputs(x, W_q, W_k, W_v, W_o, ff_w1, ff_w2, cfg)
    res = run_bass_kernel_spmd(nc, in_maps, core_ids=list(range(cfg.n_cores)))
    return gather_outputs(res.results, cfg, x.shape[0])
